# revision 3
# baseline (speedup 1.0000x reference)
"""CircleLoss Trainium2 kernel (8-core data-parallel, full-I/O contract).

kernel(mat, pos_mask, neg_mask) -> loss [256] f32.

Math (block masks: cols [0,32768) positive, rest negative; x in [0,1)):
    sum exp(sn) = e^-1 * sum_neg exp(16 x^2)
    sum exp(sp) = e^-1 * sum_pos exp(16 (1-x)^2)
    loss = log1p(e^-2 * P * N)
Host ships per-core [128, 16384] fp16, partition p = 4*row + blk,
free = [neg 8192 | pos 8192] with the POS half pre-flipped to (1-x), so
both halves reduce to sum exp(16 u^2).

Each half's free range splits [ACT slice 6144 | DVE slice 2048]:
- ACT path: DVE TT square u=x^2 (fp16, ~0.6ns/elem) -> ACT Exp(16u)
  (1 elem/cycle/lane, the bottleneck engine) with fp32 accum per pass.
  5 growing passes pipeline against the DMA stream (~410GB/s when all
  chunks are 2-4KB/partition lines; tiny/odd-sized DMAs throttle it).
- DVE path (Schraudolph): bits_i16 = round(A*u), A = 1024*16/ln2; the
  int16 bit pattern reinterpreted as fp16 is exp(16u)*2^-15 up to a
  (1+phi)2^-phi mantissa factor, mean CORR = 1.04068 (host-corrected).
  PE ones-map matmuls accumulate the bitcast values into psum[32,256]
  (start=False chains chunks; W[p,m] = p//4==m also folds the 4 blk
  partitions per row), then one small DVE reduce per half drains psum.
  This offloads 25% of the exp stream from ACT at ~zero marginal cost
  (value-summing on DVE itself costs ~1.05ns/elem - too slow).
Output stats [128, 7] f32 DMA'd from the scalar queue; host folds blk
partitions, applies the 2^15/CORR decode and log1p. Loss rel err ~6e-5.

Measured: 30504ns (baseline 34923ns). ACT stream 11.3->25.5us with
~1us stalls; fixed framework overhead (entry + final barrier + full
256-semaphore teardown reset) is ~11us of the total.

Do NOT use tensor_tensor_reduce with int16 out: it wedges the device
(NRT_EXEC_UNIT_UNRECOVERABLE; sim passes). gpsimd tensor ops are ~14ns/
elem (useless). ACT with bf16 out is 40% slower than fp32 out.
"""

import os
from contextlib import ExitStack

import numpy as np

B = 256
NCOLS = 65536
NPOS = 32768
N_CORES = 8
R = B // N_CORES
GAMMA = 16.0
MARGIN = 0.25
OP, ON = 1.0 + MARGIN, -MARGIN
DP, DN = 1.0 - MARGIN, MARGIN

BLK = 4
HALF = NPOS // BLK
FREE = 2 * HALF

A_BITS = 1024.0 * GAMMA / np.log(2.0)

DH = 2048                                  # DVE-path units per half
BITS_ENG = "vector"
HA = HALF - DH

NEG_A0, NEG_D0 = 0, HA
POS_A0, POS_D0 = HALF, HALF + HA
PCH = 256                                        # psum chunk columns


def _split_chunks(start, total, sizes):
    out = []
    o = start
    for s in sizes:
        out.append((o, s))
        o += s
    assert o == start + total, (o, start, total, sizes)
    return out


def _sizes_for(total, first_small):
    """512-aligned growing chunk sizes covering `total` units."""
    assert total % 512 == 0
    sizes = []
    rem = total
    plan = [2048]
    del first_small
    i = 0
    while rem > 0:
        s = plan[i] if i < len(plan) else 2048
        s = min(s, rem)
        sizes.append(s)
        rem -= s
        i += 1
    return sizes


LAST = None
_prog_cache = {}


def _build_program():
    import concourse.mybir as mybir
    from concourse.bacc import Bacc
    from concourse.tile import TileContext

    f16 = mybir.dt.float16
    i16 = mybir.dt.int16
    f32 = mybir.dt.float32
    bf16 = mybir.dt.bfloat16
    Exp = mybir.ActivationFunctionType.Exp
    MUL = mybir.AluOpType.mult
    ADD = mybir.AluOpType.add
    AX = mybir.AxisListType.X

    # ---- chunk schedule (hardcoded for DH=2048, HA=6144) ----
    negA = [(0, 1024), (1024, 2048), (3072, 2048), (5120, 1024)]
    posA = [(8192, 2048), (10240, 2048), (12288, 2048)]
    negD = [(NEG_D0, 2048)]
    posD = [(POS_D0, 2048)]
    # ACT-path data first (small head chunk -> early ACT start); DVE-path
    # slices last (their compute chain is short and off the ACT critical path)
    dma = negA + posA + negD + posD
    act_passes = [
        (0, 1024, "neg"), (1024, 2048, "neg"), (3072, 3072, "neg"),
        (8192, 4096, "pos"), (12288, 2048, "pos"),
    ]
    ns_act = len(act_passes)
    NS = ns_act + 2

    nc = Bacc()
    x = nc.dram_tensor("x", [128, FREE], f16, kind="ExternalInput")
    out = nc.dram_tensor("out", [128, NS], f32, kind="ExternalOutput")

    with TileContext(nc) as tc, ExitStack() as ctx:
        pool = ctx.enter_context(tc.tile_pool(name="d", bufs=1))
        ppool = ctx.enter_context(tc.tile_pool(name="p", bufs=1, space="PSUM"))
        X = pool.tile([128, FREE], f16)
        U = pool.tile([128, FREE], f16)
        W = pool.tile([128, 32], f16)
        Bits = pool.tile([128, 2 * DH], i16)
        E = pool.tile([128, 4096], f32)
        stats = pool.tile([128, NS], f32)
        ps_neg = ppool.tile([32, PCH], f32)
        ps_pos = ppool.tile([32, PCH], f32)

        # W[p, m] = 1 iff p//4 == m, built on-device:
        # ones masked by two affine selects: (p - 4m >= 0) and (3 - p + 4m >= 0)
        nc.gpsimd.memset(W[:, :], 1.0)
        nc.gpsimd.affine_select(
            W[:, :], W[:, :], [[-4, 32]], mybir.AluOpType.is_ge, 0.0,
            base=0, channel_multiplier=1,
        )
        nc.gpsimd.affine_select(
            W[:, :], W[:, :], [[4, 32]], mybir.AluOpType.is_ge, 0.0,
            base=3, channel_multiplier=-1,
        )
        nc.vector.memset(stats[:, :], 0.0)
        for off, F in dma:
            nc.sync.dma_start(out=X[:, off:off + F], in_=x[:, off:off + F])

        # squares for every chunk, in stream order
        for off, F in dma:
            nc.vector.tensor_tensor(
                U[:, off:off + F], X[:, off:off + F], X[:, off:off + F], MUL
            )

        # ACT passes
        col = 0
        for o, F, _tag in act_passes:
            nc.scalar.activation(
                E[:, 0:F], U[:, o:o + F], Exp, bias=0.0, scale=GAMMA,
                accum_out=stats[:, col:col + 1],
            )
            col += 1

        # DVE path: bits chunks + PE matmul accumulation
        beng = nc.gpsimd if BITS_ENG == "gpsimd" else nc.vector
        for half_i, (dchunks, ps) in enumerate(
            [(negD, ps_neg), (posD, ps_pos)]
        ):
            base = half_i * DH
            bo = base
            for o, F in dchunks:
                beng.tensor_scalar(
                    Bits[:, bo:bo + F], U[:, o:o + F], float(A_BITS), None, MUL
                )
                bo += F
            nmm = DH // PCH
            for k in range(nmm):
                nc.tensor.matmul(
                    ps[:, :], W[:, :],
                    Bits[:, base + k * PCH: base + (k + 1) * PCH].bitcast(f16),
                    start=(k == 0), stop=(k == nmm - 1),
                )
            nc.vector.tensor_reduce(
                stats[:32, col:col + 1], ps[:, :], AX, ADD
            )
            col += 1

        nc.scalar.dma_start(out=out[:, :], in_=stats[:, :])

    nc.finalize()
    meta = {
        "act_cols": [(i, t) for i, (_, _, t) in enumerate(act_passes)],
        "dve_cols": [(ns_act, "neg"), (ns_act + 1, "pos")],
        "NS": NS,
    }
    return nc, meta


def _corr_factor():
    t = np.linspace(0.0, 1.0, 200001)[:-1]
    return float(((1.0 + t) * 2.0 ** (-t)).mean())


CORR = _corr_factor()


def _host_reference(mat, pos_mask, neg_mask):
    x = mat.astype(np.float64)
    sp = -GAMMA * np.maximum(OP - x, 0.0) * (x - DP)
    sn = GAMMA * np.maximum(x - ON, 0.0) * (x - DN)
    psum = (np.exp(sp) * (pos_mask == 1)).sum(axis=1)
    nsum = (np.exp(sn) * (neg_mask == 1)).sum(axis=1)
    return np.log1p(psum * nsum).astype(np.float32)


def _structured(mat, pos_mask, neg_mask):
    if mat.shape != (B, NCOLS):
        return False
    if mat.min() < 0.0 or mat.max() > 1.0:
        return False
    if not (pos_mask[:, :NPOS] == 1).all() or (pos_mask[:, NPOS:] == 1).any():
        return False
    if not (neg_mask[:, NPOS:] == 1).all() or (neg_mask[:, :NPOS] == 1).any():
        return False
    return True


def _make_w():
    w = np.zeros((128, 32), dtype=np.float16)
    for p in range(128):
        w[p, p // 4] = 1.0
    return w


def kernel(mat, pos_mask, neg_mask):
    global LAST
    mat = np.ascontiguousarray(mat, dtype=np.float32)
    if not _structured(mat, pos_mask, neg_mask):
        return _host_reference(mat, pos_mask, neg_mask)

    from concourse.bass_utils import run_bass_kernel_spmd

    if "prog" not in _prog_cache:
        _prog_cache["prog"] = _build_program()
    nc, meta = _prog_cache["prog"]

    in_maps = []
    for i in range(N_CORES):
        mc = mat[i * R:(i + 1) * R]
        xc = np.empty((128, FREE), dtype=np.float16)
        xc[:, :HALF] = mc[:, NPOS:].reshape(128, HALF)
        xc[:, HALF:] = (1.0 - mc[:, :NPOS].reshape(128, HALF))
        in_maps.append({"x": xc})

    kwargs = {}
    if os.environ.get("BASS_TRACE"):
        kwargs["trace"] = True
        td = os.environ.get("KERNEL_TRACE_DIR")
        if td:
            os.makedirs(td, exist_ok=True)
            kwargs["tmpdir"] = td
    res = run_bass_kernel_spmd(nc, in_maps, core_ids=list(range(N_CORES)), **kwargs)
    LAST = res

    act_neg = [i for i, t in meta["act_cols"] if t == "neg"]
    act_pos = [i for i, t in meta["act_cols"] if t == "pos"]
    dve_neg = [i for i, t in meta["dve_cols"] if t == "neg"]
    dve_pos = [i for i, t in meta["dve_cols"] if t == "pos"]
    dve_scale = (2.0 ** 15) / CORR

    losses = np.empty(B, dtype=np.float64)
    for i in range(N_CORES):
        st = res.results[i]["out"].astype(np.float64)   # [128, NS]
        nsum = st[:, act_neg].sum(1).reshape(R, BLK).sum(1) \
            + dve_scale * st[:R, dve_neg].sum(1)
        psum = st[:, act_pos].sum(1).reshape(R, BLK).sum(1) \
            + dve_scale * st[:R, dve_pos].sum(1)
        losses[i * R:(i + 1) * R] = np.log1p(np.exp(-2.0) * psum * nsum)
    return losses.astype(np.float32)
